# revision 1
# baseline (speedup 1.0000x reference)
"""CRF loss (forward-algorithm partition + gold energy) on 8 TRN2 NeuronCores.

Strategy (data-parallel over batch, per the sharding hint):
  - batch 64 -> 8 cores x 8 local batches.
  - Forward recurrence kept in the *linear* domain: state q[t', b] with
    partition[b, t'] = ln q[t', b] + sum_k ln(m_k[b]).  One step is
    q <- E_b^T q per local batch (E = exp(scores[s,b])), computed as 8 tiny
    PE matvecs against bf16 E tiles produced by one big ScalarE exp per
    chunk of timesteps.  exp/log of the textbook logsumexp cancel between
    steps, so ScalarE only exponentiates each score element once.
  - Every 8 steps the state is renormalized by its column sum (computed with
    a ones-vector matmul; scaling broadcast across partitions with a rank-1
    matmul), and the sum is stashed; all logs are deferred to two ScalarE
    Ln instructions at the very end.
  - Gold-path energy: indirect-DMA element gather with host-precomputed flat
    indices, masked multiply-reduce on VectorE.
  - Per-core partials (final ln q, stashed renorm sums' logs, gold partial)
    are combined into the scalar loss on the host.
"""

import numpy as np

import concourse.bacc as bacc
import concourse.bass as bass
import concourse.mybir as mybir
import concourse.tile as tile
from concourse import bass_utils

S = 256
B = 64
T = 128
NCORES = 8
BL = B // NCORES  # 8 local batches per core
START_TAG = 126
END_TAG = 127
CHUNK = 4  # timesteps per score DMA + exp instruction
RENORM_START = 6
RENORM_EVERY = 8

f32 = mybir.dt.float32
bf16 = mybir.dt.bfloat16
i32 = mybir.dt.int32
u8 = mybir.dt.uint8
Exp = mybir.ActivationFunctionType.Exp
Ln = mybir.ActivationFunctionType.Ln
Alu = mybir.AluOpType


def renorm_steps(n_steps):
    return [s for s in range(RENORM_START, n_steps - 1, RENORM_EVERY)]


def build(n_steps=S):
    """Build + compile the SPMD kernel for one core's batch shard."""
    nrn = renorm_steps(n_steps)
    n_gather = -(-n_steps * BL // 128)  # gather columns (2048 idx -> [128, 16])
    nc = bacc.Bacc(
        "TRN2", target_bir_lowering=False, debug=False, num_devices=NCORES
    )
    sc = nc.dram_tensor("scores", [n_steps, T, BL, T], f32, kind="ExternalInput")
    p0 = nc.dram_tensor("p0t", [T, BL], f32, kind="ExternalInput").ap()
    mk = nc.dram_tensor("masks", [T, n_steps * BL], u8, kind="ExternalInput").ap()
    gi = nc.dram_tensor("tg_idx", [128, n_gather], i32, kind="ExternalInput").ap()
    gm = nc.dram_tensor("tg_msk", [128, n_gather], f32, kind="ExternalInput").ap()
    o_logq = nc.dram_tensor("out_logq", [T, BL], f32, kind="ExternalOutput").ap()
    o_tg = nc.dram_tensor("out_tg", [128, 1], f32, kind="ExternalOutput").ap()
    o_lnm = None
    if nrn:
        o_lnm = nc.dram_tensor(
            "out_lnm", [1, len(nrn) * BL], f32, kind="ExternalOutput"
        ).ap()

    with tile.TileContext(nc) as tc:
        _body(nc, tc, sc, p0, mk, gi, gm, o_logq, o_tg, o_lnm, n_steps, nrn)
    nc.compile()
    return nc


def _body(nc, tc, sc, p0, mk, gi, gm, o_logq, o_tg, o_lnm, n_steps, nrn):
    import os
    from contextlib import ExitStack

    nogather = os.environ.get("K_NOGATHER")
    nomasks = os.environ.get("K_NOMASKS")
    norenorm = os.environ.get("K_NORENORM")
    noexp = os.environ.get("K_NOEXP")
    nomm = os.environ.get("K_NOMM")
    repeat = int(os.environ.get("K_REPEAT", "1"))

    n_gather = gi.shape[1]
    sc_ap = sc.ap()

    with ExitStack() as ctx:
        const = ctx.enter_context(tc.tile_pool(name="const", bufs=1))
        spool = ctx.enter_context(tc.tile_pool(name="spool", bufs=3))
        epool = ctx.enter_context(tc.tile_pool(name="epool", bufs=3))
        vpool = ctx.enter_context(tc.tile_pool(name="vpool", bufs=4, space="PSUM"))
        rpool = ctx.enter_context(tc.tile_pool(name="rpool", bufs=2, space="PSUM"))
        small = ctx.enter_context(tc.tile_pool(name="small", bufs=2))

        # ---- constants & persistent state ----
        ones_col = const.tile([128, 1], bf16)
        nc.vector.memset(ones_col[:], 1.0)
        ones_row = const.tile([1, 128], f32)
        nc.vector.memset(ones_row[:], 1.0)
        q = const.tile([128, BL], bf16)  # recurrence state
        mbuf = None
        if nrn and not nomm:
            mbuf = const.tile([1, len(nrn) * BL], f32)  # stashed renorm sums
        masks_sb = const.tile([128, n_steps * BL], u8)
        nc.sync.dma_start(out=masks_sb[:], in_=mk[:])

        # ---- init: q = exp(scores[0, :, START_TAG, :]^T) ----
        p0_sb = small.tile([128, BL], f32)
        nc.sync.dma_start(out=p0_sb[:], in_=p0[:])

        # ---- gold energy gather (independent of the recurrence) ----
        if nogather:
            tgz = const.tile([128, 1], f32)
            nc.vector.memset(tgz[:], 0.0)
            nc.sync.dma_start(out=o_tg[:], in_=tgz[:])
        gidx = const.tile([128, n_gather], i32)
        if not nogather:
            nc.sync.dma_start(out=gidx[:], in_=gi[:])
        if not nogather:
            gmask = const.tile([128, n_gather], f32)
            nc.sync.dma_start(out=gmask[:], in_=gm[:])
            gath = const.tile([128, n_gather], f32)
            n_elem = n_steps * BL * T * T
            sc_flat = bass.AP(tensor=sc, offset=0, ap=[[1, n_elem], [1, 1]])
            for j in range(n_gather):
                nc.gpsimd.indirect_dma_start(
                    out=gath[:, j : j + 1],
                    out_offset=None,
                    in_=sc_flat,
                    in_offset=bass.IndirectOffsetOnAxis(ap=gidx[:, j : j + 1], axis=0),
                )
            prod = const.tile([128, n_gather], f32)
            tgc = const.tile([128, 1], f32)
            nc.vector.tensor_tensor(
                out=prod[:], in0=gath[:], in1=gmask[:], op=Alu.mult
            )
            nc.vector.reduce_sum(
                out=tgc[:], in_=prod[:], axis=mybir.AxisListType.X
            )
            nc.sync.dma_start(out=o_tg[:], in_=tgc[:])

        # ---- main recurrence over timesteps 1..n_steps-1 ----
        nrn_set = set(nrn)
        for rep in range(repeat):
            nc.scalar.activation(out=q[:], in_=p0_sb[:], func=Exp)
            k_renorm = 0
            s = 1
            while s < n_steps:
                hi = min(s + CHUNK, n_steps)
                nsub = hi - s
                # stream scores[s:hi] as [t, (s b u)] and exponentiate once
                sc_tile = spool.tile([128, nsub * BL * T], f32, tag="sc")
                nc.sync.dma_start(
                    out=sc_tile[:],
                    in_=sc_ap[s:hi].rearrange("s t b u -> t s b u"),
                )
                if noexp:
                    e_tile = sc_tile.bitcast(bf16)[:, : nsub * BL * T]
                else:
                    e_tile = epool.tile([128, nsub * BL * T], bf16, tag="e")
                    nc.scalar.activation(out=e_tile[:], in_=sc_tile[:], func=Exp)
                for sl in range(nsub):
                    step = s + sl
                    if nomm:
                        continue
                    v = vpool.tile([128, BL], f32, tag="v")
                    for b in range(BL):
                        off = (sl * BL + b) * T
                        nc.tensor.matmul(
                            out=v[:, b : b + 1],
                            lhsT=e_tile[:, off : off + T],
                            rhs=q[:, b : b + 1],
                            start=True,
                            stop=True,
                        )
                    # q <- v where mask_for_padding[step] else q
                    if nomasks:
                        nc.vector.tensor_copy(out=q[:], in_=v[:])
                    else:
                        nc.vector.copy_predicated(
                            out=q[:],
                            mask=masks_sb[:, step * BL : (step + 1) * BL],
                            data=v[:],
                        )
                    if step in nrn_set and not norenorm:
                        ssum = rpool.tile([1, BL], f32, tag="sum")
                        nc.tensor.matmul(
                            out=ssum[:],
                            lhsT=ones_col[:],
                            rhs=q[:],
                            start=True,
                            stop=True,
                        )
                        nc.vector.tensor_copy(
                            out=mbuf[:, k_renorm * BL : (k_renorm + 1) * BL],
                            in_=ssum[:],
                        )
                        r_row = small.tile([1, BL], f32, tag="rrow")
                        nc.vector.reciprocal(out=r_row[:], in_=ssum[:])
                        r_bc = rpool.tile([128, BL], f32, tag="rbc")
                        nc.tensor.matmul(
                            out=r_bc[:],
                            lhsT=ones_row[:],
                            rhs=r_row[:],
                            start=True,
                            stop=True,
                        )
                        nc.vector.tensor_tensor(
                            out=q[:], in0=q[:], in1=r_bc[:], op=Alu.mult
                        )
                        k_renorm += 1
                s = hi

        # ---- finalize ----
        logq = small.tile([128, BL], f32, tag="logq")
        nc.scalar.activation(out=logq[:], in_=q[:], func=Ln)
        nc.sync.dma_start(out=o_logq[:], in_=logq[:])
        if nrn:
            lnm_t = small.tile([1, len(nrn) * BL], f32, tag="lnm")
            if mbuf is None:
                nc.vector.memset(lnm_t[:], 0.0)
            else:
                nc.scalar.activation(out=lnm_t[:], in_=mbuf[:], func=Ln)
            nc.sync.dma_start(out=o_lnm[:], in_=lnm_t[:])


def make_in_maps(scores, target, mask_gold, mask_pad, n_steps=S):
    """Host-side sharding/preprocessing -> per-core input dicts."""
    scores = np.asarray(scores, dtype=np.float32)
    target = np.asarray(target).astype(np.int64)
    mg = np.asarray(mask_gold).astype(np.float32)
    mp = np.asarray(mask_pad).astype(np.float32)
    n_gather = -(-n_steps * BL // 128)
    in_maps = []
    for c in range(NCORES):
        b0 = c * BL
        sc_c = np.ascontiguousarray(
            scores[:n_steps, b0 : b0 + BL].transpose(0, 2, 1, 3)
        )
        p0_c = np.ascontiguousarray(scores[0, b0 : b0 + BL, START_TAG, :].T)
        mrow = mp[:n_steps, b0 : b0 + BL].reshape(-1)
        mk_c = np.ascontiguousarray(
            np.broadcast_to(mrow[None, :], (128, n_steps * BL))
        ).astype(np.uint8)
        tgt = target[:n_steps, b0 : b0 + BL, 0]
        tfrom = tgt // T
        tto = tgt % T
        sidx = (
            (
                (np.arange(n_steps, dtype=np.int64)[:, None] * T + tfrom) * BL
                + np.arange(BL, dtype=np.int64)[None, :]
            )
            * T
            + tto
        ).reshape(-1)
        gmv = mg[:n_steps, b0 : b0 + BL].reshape(-1)
        pad = n_gather * 128 - sidx.shape[0]
        if pad:
            sidx = np.concatenate([sidx, np.zeros(pad, dtype=np.int64)])
            gmv = np.concatenate([gmv, np.zeros(pad, dtype=np.float32)])
        gi_c = np.ascontiguousarray(
            sidx.reshape(n_gather, 128).T.astype(np.int32)
        )
        gm_c = np.ascontiguousarray(gmv.reshape(n_gather, 128).T)
        in_maps.append(
            {
                "scores": sc_c,
                "p0t": p0_c,
                "masks": mk_c,
                "tg_idx": gi_c,
                "tg_msk": gm_c,
            }
        )
    return in_maps


def combine(results, n_steps=S):
    """Host-side reduction of per-core partials -> scalar loss."""
    part = 0.0
    tg = 0.0
    for r in results:
        part += float(r["out_logq"][END_TAG, :].sum(dtype=np.float64))
        if "out_lnm" in r:
            part += float(r["out_lnm"].sum(dtype=np.float64))
        tg += float(r["out_tg"].sum(dtype=np.float64))
    return np.float32((part - tg) / B)


_NC_CACHE = {}


def kernel(scores, target, mask_for_gold, mask_for_padding):
    if "nc" not in _NC_CACHE:
        _NC_CACHE["nc"] = build(S)
    nc = _NC_CACHE["nc"]
    in_maps = make_in_maps(scores, target, mask_for_gold, mask_for_padding, S)
    res = bass_utils.run_bass_kernel_spmd(
        nc, in_maps, core_ids=list(range(NCORES))
    )
    return combine(res.results, S)

